# revision 43
# baseline (speedup 1.0000x reference)
"""CAM (channel attention) module kernel for Trainium2, 8 NeuronCores.

Reference computation (per batch b):
    q = x[b].reshape(C, N)                      # C=128, N=65536
    energy = q @ q.T                            # C x C
    att = softmax(rowmax(energy) - energy)      # == exp(rowmin(e)-e)/rowsum
    out = att @ q
    result = gamma * out + x

Sharding: every core takes the same N/8 = 8192 column slice of BOTH
batches.  The two batches are pipelined: batch 0's energy -> AllReduce 0
(over all 8 cores) overlaps batch 1's energy compute, and batch 0's
AV/residual/store tail overlaps AllReduce 1.  This hides most of the
collective latency behind compute.

Numerics: the PE matmuls run fp16 with an hi/lo split for the energy
term:  q = qh + ql (fp16 each, ~22 mantissa bits combined), and
    E = Qh Qh^T + C + C^T,   C = sum_j Qh_j Ql_j^T
which keeps the absolute error of the 65536-length dot products small
enough for the softmax (exp) stage.  The hi/lo transposed chunks are
interleaved in one SBUF tensor so each energy step is a single N=256
matmul accumulating [E_hh | C] into one PSUM tile.  Transposes run on
the TensorE.  The residual add uses the exact f32 copy of x.  gamma is
folded into the attention matrix, so the residual is a single add.
"""

import numpy as np

import concourse.bass as bass
import concourse.mybir as mybir
import concourse.tile as tile
from concourse import bacc
from concourse.bass_utils import run_bass_kernel_spmd
from concourse.masks import make_identity

B, C, D, H, W = 2, 128, 16, 64, 64
N = D * H * W  # 65536
NCORES = 8
NS = N // NCORES  # 8192 columns per core per batch

F32 = mybir.dt.float32
F16 = mybir.dt.float16

# tuning knobs
CFG = dict(
    nb=1024,          # pipeline block (cast/sub granularity)
    load_plan=(512, 512, 1024, 2048, 4096),
    store_nb=1024,    # output store DMA granularity
    avf=512,          # AV matmul free-dim chunk
    av_bufs=2,
    use_collective=True,
)

GROUPS = [[0, 1, 2, 3, 4, 5, 6, 7]]


def _body(nc: bass.Bass, tc: "tile.TileContext", xs, gm, out, cfg):
    NB = cfg["nb"]
    AVF = cfg["avf"]
    JCH = NS // 128          # transposed 128-chunks per batch half
    with (
        tc.tile_pool(name="big", bufs=1) as big,
        tc.tile_pool(name="small", bufs=1) as small,
        tc.tile_pool(name="work", bufs=3) as work,
        tc.tile_pool(name="qlb", bufs=3) as qlb,
        tc.tile_pool(name="psum_e", bufs=1, space="PSUM") as pse,
        tc.tile_pool(name="psum_av", bufs=cfg["av_bufs"], space="PSUM") as psav,
        tc.tile_pool(name="trps", bufs=2, space="PSUM") as trps,
        tc.tile_pool(name="dram", bufs=1, space="DRAM") as dram,
    ):
        # Persistent SBUF tensors; column range [b*NS, (b+1)*NS) = batch b
        xf = big.tile([C, 2 * NS], F32, tag="xf")      # exact f32 x
        qh = big.tile([C, 2 * NS], F16, tag="qh")      # fp16 hi (AV rhs)
        # transposed chunks, [hi_j | lo_j] interleaved along the free dim
        qT = big.tile([128, 2 * JCH, 256], F16, tag="qT")

        identh = small.tile([128, 128], F16, tag="identh")
        make_identity(nc, identh)
        ident = small.tile([128, 128], F32, tag="ident")
        make_identity(nc, ident)

        g0 = small.tile([1, 1], F32, tag="g0")
        gsb = small.tile([128, 1], F32, tag="gsb")
        nc.sync.dma_start(g0[:], gm[None, :])
        nc.gpsimd.partition_broadcast(gsb, g0[:])

        GB = 512
        gjp = GB // 128   # 4 chunks per transpose group

        ec_ps = [
            pse.tile([128, 256], F32, tag=f"ec_ps{b}", name=f"ec_ps{b}")
            for b in range(2)
        ]

        def load(b):
            pos = b * NS
            for ln in cfg["load_plan"]:
                nc.sync.dma_start(xf[:, pos:pos + ln], xs[:, pos:pos + ln])
                pos += ln
            assert pos == (b + 1) * NS

        def phase1(b):
            """split-cast -> PE-transpose -> energy MMs for batch b."""
            base = b * NS
            jbase = b * JCH

            def emit_emm(jlist):
                for j in jlist:
                    jj = jbase + j
                    nc.tensor.matmul(
                        ec_ps[b], lhsT=qT[:, jj, 0:128], rhs=qT[:, jj, :],
                        start=(j == 0), stop=(j == JCH - 1),
                    )

            nblk = NS // NB
            for blk in range(nblk):
                sl = slice(base + blk * NB, base + (blk + 1) * NB)
                nc.vector.tensor_copy(qh[:, sl], xf[:, sl])        # fp16 hi
                ql = qlb.tile([C, NB], F16, tag="ql")
                nc.vector.tensor_tensor(                            # fp16 lo
                    ql, xf[:, sl], qh[:, sl], mybir.AluOpType.subtract
                )
                for gg in range(NB // GB):
                    g = blk * (NB // GB) + gg
                    th = trps.tile([128, GB], F16, tag="th")
                    tl = trps.tile([128, GB], F16, tag="tl")
                    for u in range(gjp):
                        a0 = base + blk * NB + gg * GB + u * 128
                        r0 = gg * GB + u * 128
                        ps = slice(u * 128, (u + 1) * 128)
                        nc.tensor.transpose(th[:, ps], qh[:, a0:a0 + 128], identh)
                        nc.tensor.transpose(tl[:, ps], ql[:, r0:r0 + 128], identh)
                    jsl = slice(jbase + g * gjp, jbase + (g + 1) * gjp)
                    nc.scalar.copy(
                        qT[:, jsl, 0:128],
                        th.rearrange("p (a b) -> p a b", b=128),
                    )
                    nc.vector.tensor_copy(
                        qT[:, jsl, 128:256],
                        tl.rearrange("p (a b) -> p a b", b=128),
                    )
                    if g > 0:
                        emit_emm(range((g - 1) * gjp, g * gjp))
            emit_emm(range(JCH - gjp, JCH))

        def reduce_energy(b):
            """E = E_hh + C + C^T, then AllReduce across all 8 cores."""
            c_sb = small.tile([128, 128], F32, tag=f"c_sb{b}")
            nc.vector.tensor_copy(c_sb, ec_ps[b][:, 128:256])
            cT_ps = trps.tile([128, 128], F32, tag="th")
            nc.tensor.transpose(cT_ps, c_sb, ident)
            e_sb = small.tile([128, 128], F32, tag=f"e_sb{b}")
            nc.vector.tensor_add(e_sb, ec_ps[b][:, 0:128], c_sb)
            nc.vector.tensor_add(e_sb, e_sb, cT_ps)
            if not cfg["use_collective"]:
                return e_sb
            e_in = dram.tile([128, 128], F32, tag=f"e_in{b}")
            e_out = dram.tile([128, 128], F32, tag=f"e_out{b}")
            nc.sync.dma_start(e_in[:], e_sb)
            nc.gpsimd.collective_compute(
                "AllReduce",
                mybir.AluOpType.add,
                replica_groups=GROUPS,
                ins=[e_in.opt()],
                outs=[e_out.opt()],
            )
            e_full = small.tile([128, 128], F32, tag=f"e_full{b}")
            nc.sync.dma_start(e_full, e_out[:])
            return e_full

        def softmax_attT(b, e_full):
            """att^T (fp16, gamma folded) from the reduced energy."""
            m = small.tile([128, 1], F32, tag=f"m{b}")
            nc.vector.tensor_reduce(
                m, e_full, axis=mybir.AxisListType.X, op=mybir.AluOpType.min
            )
            t = small.tile([128, 128], F32, tag=f"t{b}")
            r = small.tile([128, 1], F32, tag=f"r{b}")
            nc.scalar.activation(
                t, e_full, mybir.ActivationFunctionType.Exp,
                bias=m, scale=-1.0, accum_out=r,
            )
            rinv = small.tile([128, 1], F32, tag=f"rinv{b}")
            nc.vector.reciprocal(rinv, r)
            att = small.tile([128, 128], F32, tag=f"att{b}")
            nc.vector.tensor_scalar(
                att, t, rinv, gsb, mybir.AluOpType.mult, mybir.AluOpType.mult
            )
            attT_ps = trps.tile([128, 128], F32, tag="th")
            nc.tensor.transpose(attT_ps, att, ident)
            attT = small.tile([128, 128], F16, tag=f"attT{b}")
            nc.vector.tensor_copy(attT, attT_ps)
            return attT

        def av_tail(b, attT):
            """AV matmul + residual + store for batch b."""
            base = b * NS
            SNB = cfg["store_nb"]
            per_store = SNB // AVF
            store_engs = [nc.sync, nc.scalar, nc.gpsimd]
            o_sb = None
            for f in range(NS // AVF):
                sl = slice(base + f * AVF, base + (f + 1) * AVF)
                rr = f % 4
                if rr == 0:
                    av_ps = psav.tile([128, AVF], F32, tag="av_ps")
                elif rr == 1:
                    av_ps = trps.tile([128, AVF], F32, tag="th")
                elif rr == 2:
                    av_ps = trps.tile([128, AVF], F32, tag="tl")
                else:
                    av_ps = pse.tile([128, AVF], F32, tag=f"ec_ps{b}",
                                     name=f"avec{b}_{f}")
                nc.tensor.matmul(av_ps, lhsT=attT, rhs=qh[:, sl],
                                 start=True, stop=True)
                if f % per_store == 0:
                    o_sb = work.tile([128, SNB], F32, tag="o_sb")
                osl = slice((f % per_store) * AVF, (f % per_store + 1) * AVF)
                if f % 8 in (2, 5, 7):
                    avs = work.tile([128, AVF], F16, tag="avs")
                    nc.scalar.copy(avs, av_ps)
                    nc.gpsimd.tensor_add(o_sb[:, osl], avs, xf[:, sl])
                else:
                    nc.vector.tensor_add(o_sb[:, osl], av_ps, xf[:, sl])
                if (f + 1) % per_store == 0:
                    st = slice(base + (f + 1 - per_store) * AVF,
                               base + (f + 1) * AVF)
                    dma_eng = store_engs[(f // per_store) % 3]
                    dma_eng.dma_start(out[:, st], o_sb)

        # ---- pipelined schedule over the two batches ----
        load(0)
        load(1)
        phase1(0)
        e0 = reduce_energy(0)      # AR0 overlaps phase1(1)
        phase1(1)
        e1 = reduce_energy(1)      # AR1 queues right behind AR0
        a0 = softmax_attT(0, e0)
        av_tail(0, a0)             # tail 0 overlaps AR1
        a1 = softmax_attT(1, e1)
        av_tail(1, a1)


_cached_nc = None


def _build(cfg=None):
    cfg = dict(CFG, **(cfg or {}))
    nc = bacc.Bacc(
        "TRN2",
        target_bir_lowering=False,
        debug=False,
        enable_asserts=False,
        num_devices=NCORES,
    )
    xs = nc.dram_tensor("xs", [C, 2 * NS], F32, kind="ExternalInput").ap()
    gm = nc.dram_tensor("gamma", [1], F32, kind="ExternalInput").ap()
    out = nc.dram_tensor("out", [C, 2 * NS], F32, kind="ExternalOutput").ap()
    with tile.TileContext(nc) as tc:
        _body(nc, tc, xs, gm, out, cfg)
    nc.compile()
    return nc


def kernel(x: np.ndarray, gamma: np.ndarray, _collect_results=None) -> np.ndarray:
    global _cached_nc
    if _cached_nc is None:
        _cached_nc = _build()
    nc = _cached_nc

    xr = np.ascontiguousarray(np.asarray(x, dtype=np.float32).reshape(B, C, N))
    gamma = np.ascontiguousarray(np.asarray(gamma, dtype=np.float32))
    in_maps = []
    for k in range(NCORES):
        shard = np.concatenate(
            [xr[0, :, k * NS:(k + 1) * NS], xr[1, :, k * NS:(k + 1) * NS]],
            axis=1,
        )
        in_maps.append({"xs": np.ascontiguousarray(shard), "gamma": gamma})

    res = run_bass_kernel_spmd(nc, in_maps, core_ids=list(range(NCORES)))
    if _collect_results is not None:
        _collect_results.append(res)

    outf = np.empty((B, C, N), np.float32)
    for k in range(NCORES):
        o = res.results[k]["out"]
        outf[0, :, k * NS:(k + 1) * NS] = o[:, :NS]
        outf[1, :, k * NS:(k + 1) * NS] = o[:, NS:]
    return outf.reshape(B, C, D, H, W)


# revision 44
# speedup vs baseline: 1.0290x; 1.0290x over previous
"""CAM (channel attention) module kernel for Trainium2, 8 NeuronCores.

Reference computation (per batch b):
    q = x[b].reshape(C, N)                      # C=128, N=65536
    energy = q @ q.T                            # C x C
    att = softmax(rowmax(energy) - energy)      # == exp(rowmin(e)-e)/rowsum
    out = att @ q
    result = gamma * out + x

Sharding: every core takes the same N/8 = 8192 column slice of BOTH
batches.  The two batches are pipelined: batch 0's energy -> AllReduce 0
(over all 8 cores) overlaps batch 1's energy compute, and batch 0's
AV/residual/store tail overlaps AllReduce 1.  This hides most of the
collective latency behind compute.

Numerics: the PE matmuls run fp16 with an hi/lo split for the energy
term:  q = qh + ql (fp16 each, ~22 mantissa bits combined), and
    E = Qh Qh^T + C + C^T,   C = sum_j Qh_j Ql_j^T
which keeps the absolute error of the 65536-length dot products small
enough for the softmax (exp) stage.  The hi/lo transposed chunks are
interleaved in one SBUF tensor so each energy step is a single N=256
matmul accumulating [E_hh | C] into one PSUM tile.  Transposes run on
the TensorE.  The residual add uses the exact f32 copy of x.  gamma is
folded into the attention matrix, so the residual is a single add.
"""

import numpy as np

import concourse.bass as bass
import concourse.mybir as mybir
import concourse.tile as tile
from concourse import bacc
from concourse.bass_utils import run_bass_kernel_spmd
from concourse.masks import make_identity

B, C, D, H, W = 2, 128, 16, 64, 64
N = D * H * W  # 65536
NCORES = 8
NS = N // NCORES  # 8192 columns per core per batch

F32 = mybir.dt.float32
F16 = mybir.dt.float16

# tuning knobs
CFG = dict(
    nb=1024,          # pipeline block (cast/sub granularity)
    load_plan=(512, 512, 1024, 2048, 4096),
    store_nb=2048,    # output store DMA granularity
    avf=512,          # AV matmul free-dim chunk
    av_bufs=2,
    use_collective=True,
)

GROUPS = [[0, 1, 2, 3, 4, 5, 6, 7]]


def _body(nc: bass.Bass, tc: "tile.TileContext", xs, gm, out, cfg):
    NB = cfg["nb"]
    AVF = cfg["avf"]
    JCH = NS // 128          # transposed 128-chunks per batch half
    with (
        tc.tile_pool(name="big", bufs=1) as big,
        tc.tile_pool(name="small", bufs=1) as small,
        tc.tile_pool(name="work", bufs=3) as work,
        tc.tile_pool(name="qlb", bufs=3) as qlb,
        tc.tile_pool(name="psum_e", bufs=1, space="PSUM") as pse,
        tc.tile_pool(name="psum_av", bufs=cfg["av_bufs"], space="PSUM") as psav,
        tc.tile_pool(name="trps", bufs=2, space="PSUM") as trps,
        tc.tile_pool(name="dram", bufs=1, space="DRAM") as dram,
    ):
        # Persistent SBUF tensors; column range [b*NS, (b+1)*NS) = batch b
        xf = big.tile([C, 2 * NS], F32, tag="xf")      # exact f32 x
        qh = big.tile([C, 2 * NS], F16, tag="qh")      # fp16 hi (AV rhs)
        # transposed chunks, [hi_j | lo_j] interleaved along the free dim
        qT = big.tile([128, 2 * JCH, 256], F16, tag="qT")

        identh = small.tile([128, 128], F16, tag="identh")
        make_identity(nc, identh)
        ident = small.tile([128, 128], F32, tag="ident")
        make_identity(nc, ident)

        g0 = small.tile([1, 1], F32, tag="g0")
        gsb = small.tile([128, 1], F32, tag="gsb")
        nc.sync.dma_start(g0[:], gm[None, :])
        nc.gpsimd.partition_broadcast(gsb, g0[:])

        GB = 512
        gjp = GB // 128   # 4 chunks per transpose group

        ec_ps = [
            pse.tile([128, 256], F32, tag=f"ec_ps{b}", name=f"ec_ps{b}")
            for b in range(2)
        ]

        def load(b):
            pos = b * NS
            for ln in cfg["load_plan"]:
                nc.sync.dma_start(xf[:, pos:pos + ln], xs[:, pos:pos + ln])
                pos += ln
            assert pos == (b + 1) * NS

        def phase1(b):
            """split-cast -> PE-transpose -> energy MMs for batch b."""
            base = b * NS
            jbase = b * JCH

            def emit_emm(jlist):
                for j in jlist:
                    jj = jbase + j
                    nc.tensor.matmul(
                        ec_ps[b], lhsT=qT[:, jj, 0:128], rhs=qT[:, jj, :],
                        start=(j == 0), stop=(j == JCH - 1),
                    )

            nblk = NS // NB
            for blk in range(nblk):
                sl = slice(base + blk * NB, base + (blk + 1) * NB)
                nc.vector.tensor_copy(qh[:, sl], xf[:, sl])        # fp16 hi
                ql = qlb.tile([C, NB], F16, tag="ql")
                nc.vector.tensor_tensor(                            # fp16 lo
                    ql, xf[:, sl], qh[:, sl], mybir.AluOpType.subtract
                )
                for gg in range(NB // GB):
                    g = blk * (NB // GB) + gg
                    th = trps.tile([128, GB], F16, tag="th")
                    tl = trps.tile([128, GB], F16, tag="tl")
                    for u in range(gjp):
                        a0 = base + blk * NB + gg * GB + u * 128
                        r0 = gg * GB + u * 128
                        ps = slice(u * 128, (u + 1) * 128)
                        nc.tensor.transpose(th[:, ps], qh[:, a0:a0 + 128], identh)
                        nc.tensor.transpose(tl[:, ps], ql[:, r0:r0 + 128], identh)
                    jsl = slice(jbase + g * gjp, jbase + (g + 1) * gjp)
                    nc.scalar.copy(
                        qT[:, jsl, 0:128],
                        th.rearrange("p (a b) -> p a b", b=128),
                    )
                    nc.vector.tensor_copy(
                        qT[:, jsl, 128:256],
                        tl.rearrange("p (a b) -> p a b", b=128),
                    )
                    if g > 0:
                        emit_emm(range((g - 1) * gjp, g * gjp))
            emit_emm(range(JCH - gjp, JCH))

        def reduce_energy(b):
            """E = E_hh + C + C^T, then AllReduce across all 8 cores."""
            c_sb = small.tile([128, 128], F32, tag=f"c_sb{b}")
            nc.vector.tensor_copy(c_sb, ec_ps[b][:, 128:256])
            cT_ps = trps.tile([128, 128], F32, tag="th")
            nc.tensor.transpose(cT_ps, c_sb, ident)
            e_sb = small.tile([128, 128], F32, tag=f"e_sb{b}")
            nc.vector.tensor_add(e_sb, ec_ps[b][:, 0:128], c_sb)
            nc.vector.tensor_add(e_sb, e_sb, cT_ps)
            if not cfg["use_collective"]:
                return e_sb
            e_in = dram.tile([128, 128], F32, tag=f"e_in{b}")
            e_out = dram.tile([128, 128], F32, tag=f"e_out{b}")
            nc.sync.dma_start(e_in[:], e_sb)
            nc.gpsimd.collective_compute(
                "AllReduce",
                mybir.AluOpType.add,
                replica_groups=GROUPS,
                ins=[e_in.opt()],
                outs=[e_out.opt()],
            )
            e_full = small.tile([128, 128], F32, tag=f"e_full{b}")
            nc.sync.dma_start(e_full, e_out[:])
            return e_full

        def softmax_attT(b, e_full):
            """att^T (fp16, gamma folded) from the reduced energy."""
            m = small.tile([128, 1], F32, tag=f"m{b}")
            nc.vector.tensor_reduce(
                m, e_full, axis=mybir.AxisListType.X, op=mybir.AluOpType.min
            )
            t = small.tile([128, 128], F32, tag=f"t{b}")
            r = small.tile([128, 1], F32, tag=f"r{b}")
            nc.scalar.activation(
                t, e_full, mybir.ActivationFunctionType.Exp,
                bias=m, scale=-1.0, accum_out=r,
            )
            rinv = small.tile([128, 1], F32, tag=f"rinv{b}")
            nc.vector.reciprocal(rinv, r)
            att = small.tile([128, 128], F32, tag=f"att{b}")
            nc.vector.tensor_scalar(
                att, t, rinv, gsb, mybir.AluOpType.mult, mybir.AluOpType.mult
            )
            attT_ps = trps.tile([128, 128], F32, tag="th")
            nc.tensor.transpose(attT_ps, att, ident)
            attT = small.tile([128, 128], F16, tag=f"attT{b}")
            nc.vector.tensor_copy(attT, attT_ps)
            return attT

        def av_tail(b, attT):
            """AV matmul + residual + store for batch b."""
            base = b * NS
            SNB = cfg["store_nb"]
            per_store = SNB // AVF
            store_engs = [nc.sync, nc.scalar, nc.gpsimd]
            o_sb = None
            for f in range(NS // AVF):
                sl = slice(base + f * AVF, base + (f + 1) * AVF)
                rr = f % 4
                if rr == 0:
                    av_ps = psav.tile([128, AVF], F32, tag="av_ps")
                elif rr == 1:
                    av_ps = trps.tile([128, AVF], F32, tag="th")
                elif rr == 2:
                    av_ps = trps.tile([128, AVF], F32, tag="tl")
                else:
                    av_ps = pse.tile([128, AVF], F32, tag=f"ec_ps{b}",
                                     name=f"avec{b}_{f}")
                nc.tensor.matmul(av_ps, lhsT=attT, rhs=qh[:, sl],
                                 start=True, stop=True)
                if f % per_store == 0:
                    o_sb = work.tile([128, SNB], F32, tag="o_sb")
                osl = slice((f % per_store) * AVF, (f % per_store + 1) * AVF)
                if f % 8 in (2, 5, 7):
                    avs = work.tile([128, AVF], F16, tag="avs")
                    nc.scalar.copy(avs, av_ps)
                    nc.gpsimd.tensor_add(o_sb[:, osl], avs, xf[:, sl])
                else:
                    nc.vector.tensor_add(o_sb[:, osl], av_ps, xf[:, sl])
                if (f + 1) % per_store == 0:
                    st = slice(base + (f + 1 - per_store) * AVF,
                               base + (f + 1) * AVF)
                    dma_eng = store_engs[(f // per_store) % 3]
                    dma_eng.dma_start(out[:, st], o_sb)

        # ---- pipelined schedule over the two batches ----
        load(0)
        load(1)
        phase1(0)
        e0 = reduce_energy(0)      # AR0 overlaps phase1(1)
        phase1(1)
        e1 = reduce_energy(1)      # AR1 queues right behind AR0
        a0 = softmax_attT(0, e0)
        av_tail(0, a0)             # tail 0 overlaps AR1
        a1 = softmax_attT(1, e1)
        av_tail(1, a1)


_cached_nc = None


def _build(cfg=None):
    cfg = dict(CFG, **(cfg or {}))
    nc = bacc.Bacc(
        "TRN2",
        target_bir_lowering=False,
        debug=False,
        enable_asserts=False,
        num_devices=NCORES,
    )
    xs = nc.dram_tensor("xs", [C, 2 * NS], F32, kind="ExternalInput").ap()
    gm = nc.dram_tensor("gamma", [1], F32, kind="ExternalInput").ap()
    out = nc.dram_tensor("out", [C, 2 * NS], F32, kind="ExternalOutput").ap()
    with tile.TileContext(nc) as tc:
        _body(nc, tc, xs, gm, out, cfg)
    nc.compile()
    return nc


def kernel(x: np.ndarray, gamma: np.ndarray, _collect_results=None) -> np.ndarray:
    global _cached_nc
    if _cached_nc is None:
        _cached_nc = _build()
    nc = _cached_nc

    xr = np.ascontiguousarray(np.asarray(x, dtype=np.float32).reshape(B, C, N))
    gamma = np.ascontiguousarray(np.asarray(gamma, dtype=np.float32))
    in_maps = []
    for k in range(NCORES):
        shard = np.concatenate(
            [xr[0, :, k * NS:(k + 1) * NS], xr[1, :, k * NS:(k + 1) * NS]],
            axis=1,
        )
        in_maps.append({"xs": np.ascontiguousarray(shard), "gamma": gamma})

    res = run_bass_kernel_spmd(nc, in_maps, core_ids=list(range(NCORES)))
    if _collect_results is not None:
        _collect_results.append(res)

    outf = np.empty((B, C, N), np.float32)
    for k in range(NCORES):
        o = res.results[k]["out"]
        outf[0, :, k * NS:(k + 1) * NS] = o[:, :NS]
        outf[1, :, k * NS:(k + 1) * NS] = o[:, NS:]
    return outf.reshape(B, C, D, H, W)


# revision 46
# speedup vs baseline: 1.0505x; 1.0208x over previous
"""CAM (channel attention) module kernel for Trainium2, 8 NeuronCores.

Reference computation (per batch b):
    q = x[b].reshape(C, N)                      # C=128, N=65536
    energy = q @ q.T                            # C x C
    att = softmax(rowmax(energy) - energy)      # == exp(rowmin(e)-e)/rowsum
    out = att @ q
    result = gamma * out + x

Sharding: every core takes the same N/8 = 8192 column slice of BOTH
batches.  The two batches are pipelined: batch 0's energy -> AllReduce 0
(over all 8 cores) overlaps batch 1's energy compute, and batch 0's
AV/residual/store tail overlaps AllReduce 1.  This hides most of the
collective latency behind compute.

Numerics: the PE matmuls run fp16 with an hi/lo split for the energy
term:  q = qh + ql (fp16 each, ~22 mantissa bits combined), and
    E = Qh Qh^T + C + C^T,   C = sum_j Qh_j Ql_j^T
which keeps the absolute error of the 65536-length dot products small
enough for the softmax (exp) stage.  The hi/lo transposed chunks are
interleaved in one SBUF tensor so each energy step is a single N=256
matmul accumulating [E_hh | C] into one PSUM tile.  Transposes run on
the TensorE.  The residual add uses the exact f32 copy of x.  gamma is
folded into the attention matrix, so the residual is a single add.
"""

import numpy as np

import concourse.bass as bass
import concourse.mybir as mybir
import concourse.tile as tile
from concourse import bacc
from concourse.bass_utils import run_bass_kernel_spmd
from concourse.masks import make_identity

B, C, D, H, W = 2, 128, 16, 64, 64
N = D * H * W  # 65536
NCORES = 8
NS = N // NCORES  # 8192 columns per core per batch

F32 = mybir.dt.float32
F16 = mybir.dt.float16

# tuning knobs
CFG = dict(
    nb=1024,          # pipeline block (cast/sub granularity)
    load_plan=(512, 512, 1024, 2048, 4096),
    store_nb=2048,    # output store DMA granularity
    avf=512,          # AV matmul free-dim chunk
    av_bufs=2,
    use_collective=True,
)

GROUPS = [[0, 1, 2, 3, 4, 5, 6, 7]]


def _body(nc: bass.Bass, tc: "tile.TileContext", xs, gm, out, cfg):
    NB = cfg["nb"]
    AVF = cfg["avf"]
    JCH = NS // 128          # transposed 128-chunks per batch half
    with (
        tc.tile_pool(name="big", bufs=1) as big,
        tc.tile_pool(name="small", bufs=1) as small,
        tc.tile_pool(name="work", bufs=3) as work,
        tc.tile_pool(name="qlb", bufs=3) as qlb,
        tc.tile_pool(name="psum_e", bufs=1, space="PSUM") as pse,
        tc.tile_pool(name="psum_av", bufs=cfg["av_bufs"], space="PSUM") as psav,
        tc.tile_pool(name="trps", bufs=2, space="PSUM") as trps,
        tc.tile_pool(name="dram", bufs=1, space="DRAM") as dram,
    ):
        # Persistent SBUF tensors; column range [b*NS, (b+1)*NS) = batch b
        xf = big.tile([C, 2 * NS], F32, tag="xf")      # exact f32 x
        qh = big.tile([C, 2 * NS], F16, tag="qh")      # fp16 hi (AV rhs)
        # transposed chunks, [hi_j | lo_j] interleaved along the free dim
        qT = big.tile([128, 2 * JCH, 256], F16, tag="qT")

        identh = small.tile([128, 128], F16, tag="identh")
        make_identity(nc, identh)
        ident = small.tile([128, 128], F32, tag="ident")
        make_identity(nc, ident)

        g0 = small.tile([1, 1], F32, tag="g0")
        gsb = small.tile([128, 1], F32, tag="gsb")
        nc.sync.dma_start(g0[:], gm[None, :])
        nc.gpsimd.partition_broadcast(gsb, g0[:])

        GB = 512
        gjp = GB // 128   # 4 chunks per transpose group

        ec_ps = [
            pse.tile([128, 256], F32, tag=f"ec_ps{b}", name=f"ec_ps{b}")
            for b in range(2)
        ]

        def load(b):
            pos = b * NS
            for ln in cfg["load_plan"]:
                nc.sync.dma_start(xf[:, pos:pos + ln], xs[:, pos:pos + ln])
                pos += ln
            assert pos == (b + 1) * NS

        def phase1(b):
            """split-cast -> PE-transpose -> energy MMs for batch b."""
            base = b * NS
            jbase = b * JCH

            def emit_emm(jlist):
                for j in jlist:
                    jj = jbase + j
                    nc.tensor.matmul(
                        ec_ps[b], lhsT=qT[:, jj, 0:128], rhs=qT[:, jj, :],
                        start=(j == 0), stop=(j == JCH - 1),
                    )

            nblk = NS // NB
            for blk in range(nblk):
                sl = slice(base + blk * NB, base + (blk + 1) * NB)
                nc.vector.tensor_copy(qh[:, sl], xf[:, sl])        # fp16 hi
                ql = qlb.tile([C, NB], F16, tag="ql")
                nc.vector.tensor_tensor(                            # fp16 lo
                    ql, xf[:, sl], qh[:, sl], mybir.AluOpType.subtract
                )
                for gg in range(NB // GB):
                    g = blk * (NB // GB) + gg
                    th = trps.tile([128, GB], F16, tag="th")
                    tl = trps.tile([128, GB], F16, tag="tl")
                    for u in range(gjp):
                        a0 = base + blk * NB + gg * GB + u * 128
                        r0 = gg * GB + u * 128
                        ps = slice(u * 128, (u + 1) * 128)
                        nc.tensor.transpose(th[:, ps], qh[:, a0:a0 + 128], identh)
                        nc.tensor.transpose(tl[:, ps], ql[:, r0:r0 + 128], identh)
                    jsl = slice(jbase + g * gjp, jbase + (g + 1) * gjp)
                    nc.scalar.copy(
                        qT[:, jsl, 0:128],
                        th.rearrange("p (a b) -> p a b", b=128),
                    )
                    nc.vector.tensor_copy(
                        qT[:, jsl, 128:256],
                        tl.rearrange("p (a b) -> p a b", b=128),
                    )
                    if g > 0:
                        emit_emm(range((g - 1) * gjp, g * gjp))
            emit_emm(range(JCH - gjp, JCH))

        def reduce_energy(b):
            """E = E_hh + C + C^T, then AllReduce across all 8 cores."""
            c_sb = small.tile([128, 128], F32, tag=f"c_sb{b}")
            nc.vector.tensor_copy(c_sb, ec_ps[b][:, 128:256])
            cT_ps = trps.tile([128, 128], F32, tag="th")
            nc.tensor.transpose(cT_ps, c_sb, ident)
            e_sb = small.tile([128, 128], F32, tag=f"e_sb{b}")
            nc.vector.tensor_add(e_sb, ec_ps[b][:, 0:128], c_sb)
            nc.vector.tensor_add(e_sb, e_sb, cT_ps)
            if not cfg["use_collective"]:
                return e_sb
            e_in = dram.tile([128, 128], F32, tag=f"e_in{b}")
            e_out = dram.tile([128, 128], F32, tag=f"e_out{b}")
            nc.sync.dma_start(e_in[:], e_sb)
            nc.gpsimd.collective_compute(
                "AllReduce",
                mybir.AluOpType.add,
                replica_groups=GROUPS,
                ins=[e_in.opt()],
                outs=[e_out.opt()],
            )
            e_full = small.tile([128, 128], F32, tag=f"e_full{b}")
            nc.sync.dma_start(e_full, e_out[:])
            return e_full

        def softmax_attT(b, e_full):
            """att^T (fp16, gamma folded) from the reduced energy."""
            m = small.tile([128, 1], F32, tag=f"m{b}")
            nc.vector.tensor_reduce(
                m, e_full, axis=mybir.AxisListType.X, op=mybir.AluOpType.min
            )
            t = small.tile([128, 128], F32, tag=f"t{b}")
            r = small.tile([128, 1], F32, tag=f"r{b}")
            nc.scalar.activation(
                t, e_full, mybir.ActivationFunctionType.Exp,
                bias=m, scale=-1.0, accum_out=r,
            )
            rinv = small.tile([128, 1], F32, tag=f"rinv{b}")
            nc.vector.reciprocal(rinv, r)
            att = small.tile([128, 128], F16, tag=f"att{b}")
            nc.vector.tensor_scalar(
                att, t, rinv, gsb, mybir.AluOpType.mult, mybir.AluOpType.mult
            )
            attT_ps = trps.tile([128, 128], F16, tag="th", name=f"attT_ps{b}")
            nc.tensor.transpose(attT_ps, att, identh)
            attT = small.tile([128, 128], F16, tag=f"attT{b}")
            nc.scalar.copy(attT, attT_ps)
            return attT

        def av_tail(b, attT):
            """AV matmul + residual + store for batch b."""
            base = b * NS
            SNB = cfg["store_nb"]
            per_store = SNB // AVF
            store_engs = [nc.sync, nc.scalar, nc.gpsimd]
            o_sb = None
            for f in range(NS // AVF):
                sl = slice(base + f * AVF, base + (f + 1) * AVF)
                rr = f % 4
                if rr == 0:
                    av_ps = psav.tile([128, AVF], F32, tag="av_ps")
                elif rr == 1:
                    av_ps = trps.tile([128, AVF], F32, tag="th")
                elif rr == 2:
                    av_ps = trps.tile([128, AVF], F32, tag="tl")
                else:
                    av_ps = pse.tile([128, AVF], F32, tag=f"ec_ps{b}",
                                     name=f"avec{b}_{f}")
                nc.tensor.matmul(av_ps, lhsT=attT, rhs=qh[:, sl],
                                 start=True, stop=True)
                if f % per_store == 0:
                    o_sb = work.tile([128, SNB], F32, tag="o_sb")
                osl = slice((f % per_store) * AVF, (f % per_store + 1) * AVF)
                if f % 8 in (2, 5, 7):
                    avs = work.tile([128, AVF], F16, tag="avs")
                    nc.scalar.copy(avs, av_ps)
                    nc.gpsimd.tensor_add(o_sb[:, osl], avs, xf[:, sl])
                else:
                    nc.vector.tensor_add(o_sb[:, osl], av_ps, xf[:, sl])
                if (f + 1) % per_store == 0:
                    lo = (f + 1 - per_store) * AVF
                    hi = (f + 1) * AVF
                    if f + 1 == NS // AVF:
                        # split the final store so the tail latency after
                        # the last residual add is one 1MB transfer
                        mid = (lo + hi) // 2
                        nc.sync.dma_start(
                            out[:, base + lo:base + mid], o_sb[:, 0:mid - lo])
                        nc.scalar.dma_start(
                            out[:, base + mid:base + hi], o_sb[:, mid - lo:hi - lo])
                    else:
                        dma_eng = store_engs[(f // per_store) % 3]
                        dma_eng.dma_start(out[:, base + lo:base + hi], o_sb)

        # ---- pipelined schedule over the two batches ----
        load(0)
        load(1)
        phase1(0)
        e0 = reduce_energy(0)      # AR0 overlaps phase1(1)
        phase1(1)
        e1 = reduce_energy(1)      # AR1 queues right behind AR0
        a0 = softmax_attT(0, e0)
        av_tail(0, a0)             # tail 0 overlaps AR1
        a1 = softmax_attT(1, e1)
        av_tail(1, a1)


_cached_nc = None


def _build(cfg=None):
    cfg = dict(CFG, **(cfg or {}))
    nc = bacc.Bacc(
        "TRN2",
        target_bir_lowering=False,
        debug=False,
        enable_asserts=False,
        num_devices=NCORES,
    )
    xs = nc.dram_tensor("xs", [C, 2 * NS], F32, kind="ExternalInput").ap()
    gm = nc.dram_tensor("gamma", [1], F32, kind="ExternalInput").ap()
    out = nc.dram_tensor("out", [C, 2 * NS], F32, kind="ExternalOutput").ap()
    with tile.TileContext(nc) as tc:
        _body(nc, tc, xs, gm, out, cfg)
    nc.compile()
    return nc


def kernel(x: np.ndarray, gamma: np.ndarray, _collect_results=None) -> np.ndarray:
    global _cached_nc
    if _cached_nc is None:
        _cached_nc = _build()
    nc = _cached_nc

    xr = np.ascontiguousarray(np.asarray(x, dtype=np.float32).reshape(B, C, N))
    gamma = np.ascontiguousarray(np.asarray(gamma, dtype=np.float32))
    in_maps = []
    for k in range(NCORES):
        shard = np.concatenate(
            [xr[0, :, k * NS:(k + 1) * NS], xr[1, :, k * NS:(k + 1) * NS]],
            axis=1,
        )
        in_maps.append({"xs": np.ascontiguousarray(shard), "gamma": gamma})

    res = run_bass_kernel_spmd(nc, in_maps, core_ids=list(range(NCORES)))
    if _collect_results is not None:
        _collect_results.append(res)

    outf = np.empty((B, C, N), np.float32)
    for k in range(NCORES):
        o = res.results[k]["out"]
        outf[0, :, k * NS:(k + 1) * NS] = o[:, :NS]
        outf[1, :, k * NS:(k + 1) * NS] = o[:, NS:]
    return outf.reshape(B, C, D, H, W)
